# revision 28
# baseline (speedup 1.0000x reference)
"""Trainium2 Bass kernel for a relational GCN layer — src-sharded H + AllGather.

Math (reference):
  S = feat[src]; msgs[e] = edge_nn(S[e], W_rel[rel[e]]) (tied 2-layer relu MLP)
  agg = segment_sum(msgs, dst, N); hn = LSTM-step(agg); out = MLP(hn)

Messages depend only on (rel, src): H[r, s] = edge_nn(feat[s], W_rel[r]) has
NUM_REL*N = 20k rows << E = 320k.  agg[n] = sum_{r,s} C[n, (r,s)] * H[(r,s)]
where C is the per-node edge-count matrix (dense fp8, exact small ints).

v2 layout: phase A (the H table) is sharded by SOURCE range across the 8
cores (each core computes 2*1280 = 2560 rows = 8.6us of PE instead of the
8x-duplicated 69us), then the shards are exchanged with two AllGather
collectives (one per relation) through DRAM bounce buffers and re-loaded
into SBUF.  The aggregation itself stays dst-sharded exactly as before:
core c owns dst in [1250c, 1250(c+1)) and streams its dense count matrix
C [20480, 1250] from HBM while accumulating aggT = H^T @ C on the PE.
"""

import numpy as np
import ml_dtypes

import concourse.bacc as bacc
import concourse.bass as bass
import concourse.mybir as mybir
import concourse.tile as tile
from concourse import bass_utils
from concourse.masks import make_identity

# ---- problem constants (hardcoded per spec) ----
N_NODES = 10000
N_EDGES = 320000
D = 256
D_OUT = 256
NUM_REL = 2
NCORES = 8
NPC = N_NODES // NCORES          # 1250 dst nodes per core
NBLK = 10                        # dst blocks per core (last has 98 rows)
NDST = NPC                       # exact dst cols per core (no padding)
NPAD = 10240                     # src nodes padded to 8*1280
SRC_PC = NPAD // NCORES          # 1280 src rows per core
KT_PC = NUM_REL * SRC_PC // 128  # 20 local H k-tiles per core
KT_G = KT_PC // NUM_REL          # 10 k-tiles per relation group
NROWS = NUM_REL * NPAD           # 20480 H-table rows
NKT = NROWS // 128               # 160 k-tiles
GK = 4                           # k-tiles per C DMA tile (5000B lines)
CHUNKS = [(0, 512), (512, 512), (1024, 226)]  # dst col chunks (sum 1250)
A_CHUNKS = [(0, 512), (512, 512), (1024, 256)]  # src col chunks (sum 1280)
NDST_PAD = 1280                  # aggT sbuf width (phase C uses 128-blocks)

f32 = mybir.dt.float32
f32r = mybir.dt.float32r
bf16 = mybir.dt.bfloat16
fp8 = mybir.dt.float8e4

_np_bf16 = ml_dtypes.bfloat16
_np_fp8 = ml_dtypes.float8_e4m3


def _slot(r, s):
    """SBUF hbuf slot index for H row (r, s) after the gather."""
    core = s // SRC_PC
    lkt = (s % SRC_PC) // 128
    return r * (NCORES * KT_G) + core * KT_G + lkt


# ----------------------------------------------------------------------------
# host-side preprocessing
# ----------------------------------------------------------------------------

def _prep_counts(src, dst, rel):
    """Per-core dense count matrix in fp8, grouped GK k-tiles per DMA line.

    Returns per-core arrays of shape [NKT//GK, 128, GK, NDST].
    """
    r = rel.astype(np.int64)
    s = src.astype(np.int64)
    slot = r * (NCORES * KT_G) + (s // SRC_PC) * KT_G + (s % SRC_PC) // 128
    row = slot * 128 + (s % 128)
    core = dst // NPC
    col = (dst % NPC).astype(np.int64)
    counts = np.zeros((NCORES, NROWS, NDST), dtype=np.uint8)
    np.add.at(counts, (core, row, col), 1)
    out = []
    for c in range(NCORES):
        cc = counts[c].reshape(NKT // GK, GK, 128, NDST).transpose(0, 2, 1, 3)
        out.append(np.ascontiguousarray(cc).astype(_np_fp8))
    return out


def _prep_weights(inputs):
    feat = np.asarray(inputs["feat"], dtype=np.float32)
    W_rel = np.asarray(inputs["W_rel"], dtype=np.float32)
    b_rel = np.asarray(inputs["b_rel"], dtype=np.float32)
    W_ih = np.asarray(inputs["W_ih"], dtype=np.float32)
    b_ih = np.asarray(inputs["b_ih"], dtype=np.float32)
    b_hh = np.asarray(inputs["b_hh"], dtype=np.float32)
    W1 = np.asarray(inputs["W1"], dtype=np.float32)
    W2 = np.asarray(inputs["W2"], dtype=np.float32)
    W3 = np.asarray(inputs["W3"], dtype=np.float32)

    featT = np.zeros((D, NPAD), dtype=np.float32)
    featT[:, :N_NODES] = feat.T
    keep = np.r_[0:256, 512:1024]  # i, g, o gate columns (f unused: c0 = 0)

    # bf16 pack: W_rT (4x[128,256]) | W1T (2x[128,128]) | W2T | W3T [128,256]
    W_rT = np.transpose(W_rel, (0, 2, 1))  # [rel, in, out]
    pack_b = np.concatenate(
        [W_rT[0, 0:128], W_rT[0, 128:256], W_rT[1, 0:128], W_rT[1, 128:256],
         W1.T[0:128], W1.T[128:256], W2.T, W3.T], axis=1)  # [128, 1664]
    # f32 pack: b_r_col 4x[128,1] | b1|b2|b3a|b3b 4x[128,1] |
    #           b_r_rep 2x[128,256] | b_g_rep [128,768]
    b_r_col = b_rel.reshape(NUM_REL * 2, 128, 1)
    b3 = np.asarray(inputs["b3"]).reshape(2, 128, 1)
    pack_f = np.concatenate(
        [b_r_col[0], b_r_col[1], b_r_col[2], b_r_col[3],
         np.asarray(inputs["b1"]).reshape(128, 1),
         np.asarray(inputs["b2"]).reshape(128, 1), b3[0], b3[1],
         np.broadcast_to(b_rel[0][None, :], (128, D)),
         np.broadcast_to(b_rel[1][None, :], (128, D)),
         np.broadcast_to((b_ih + b_hh)[keep][None, :], (128, 768))],
        axis=1).astype(np.float32)  # [128, 1288]

    com = {
        "pack_b": np.ascontiguousarray(pack_b).astype(_np_bf16),
        "pack_f": np.ascontiguousarray(pack_f),
        "W_ihT": np.ascontiguousarray(W_ih.T[:, keep]).astype(np.float32),
    }
    featT_b = featT.astype(_np_bf16)
    slices = []
    for c in range(NCORES):
        slices.append(np.ascontiguousarray(
            featT_b[:, c * SRC_PC:(c + 1) * SRC_PC]))
    return com, slices


# ----------------------------------------------------------------------------
# kernel builder
# ----------------------------------------------------------------------------

def _build():
    Relu = mybir.ActivationFunctionType.Relu
    Sig = mybir.ActivationFunctionType.Sigmoid
    Tanh = mybir.ActivationFunctionType.Tanh

    nc = bacc.Bacc("TRN2", target_bir_lowering=False, debug=False)

    fts_d = nc.dram_tensor("fts", [D, SRC_PC], bf16, kind="ExternalInput")
    pack_b_d = nc.dram_tensor("pack_b", [128, 1664], bf16, kind="ExternalInput")
    pack_f_d = nc.dram_tensor("pack_f", [128, 1288], f32, kind="ExternalInput")
    W_ihT_d = nc.dram_tensor("W_ihT", [D, 768], f32r, kind="ExternalInput")
    C_d = nc.dram_tensor("C", [NKT // GK, 128, GK, NDST], fp8,
                         kind="ExternalInput")
    outT_d = nc.dram_tensor("outT", [D_OUT, NPC], f32, kind="ExternalOutput")

    with tile.TileContext(nc) as tc:
        with (
            tc.tile_pool(name="const", bufs=1) as cp,
            tc.tile_pool(name="work", bufs=3) as wp,
            tc.tile_pool(name="hbig", bufs=1) as hp_pool,
            tc.tile_pool(name="aggpool", bufs=1) as ap_pool,
            tc.tile_pool(name="dram", bufs=1, space="DRAM") as dram,
        ):
            # DRAM bounce buffers for the H-shard exchange; the gather
            # output is Shared so the AllGather takes the fast direct-write
            # path instead of the slow mesh algorithm.
            shard_d = [dram.tile([128, KT_G, D], bf16, tag=f"shard{g}",
                                 name=f"shard{g}")
                       for g in range(NUM_REL)]
            gath_d = [dram.tile([NCORES, 128, KT_G, D], bf16, tag=f"gath{g}",
                                name=f"gath{g}", addr_space="Shared")
                      for g in range(NUM_REL)]
            # Tiny warm-up AllGather: pays the collective channel setup and
            # inter-core rendezvous cost while the constants stream in, so
            # the real H-shard AllGathers start without it.
            warm_in = dram.tile([128, 4], f32, tag="warm_in", name="warm_in")
            warm_out = dram.tile([NCORES, 128, 4], f32, tag="warm_out",
                                 name="warm_out")
            warm_sb = cp.tile([128, 4], f32, tag="warm_sb")
            nc.gpsimd.memset(warm_sb[:], 0.0)
            nc.gpsimd.dma_start(warm_in[:, :], warm_sb[:])
            nc.gpsimd.collective_compute(
                "AllGather",
                mybir.AluOpType.bypass,
                replica_groups=[list(range(NCORES))],
                ins=[warm_in[:, :].opt()],
                outs=[warm_out[:, :, :].opt()],
            )

            # ---- constants, ordered so phase A's first chunk starts ASAP:
            # W_r0 weights + first feat chunk on the two HW queues first.
            pb = cp.tile([128, 1664], bf16, tag="pb")
            nc.sync.dma_start(pb[:, 0:512], pack_b_d[:, 0:512])
            pf = cp.tile([128, 1288], f32, tag="pf")
            nc.scalar.dma_start(pf[:, 0:8], pack_f_d[:, 0:8])
            ft = {}
            for h in range(2):
                t = cp.tile([128, SRC_PC], bf16, tag=f"ft{h}")
                ft[h] = t
            for ci, (c0, cw) in enumerate(A_CHUNKS):
                for h in range(2):
                    eng = nc.sync if (ci * 2 + h) % 2 == 0 else nc.scalar
                    eng.dma_start(ft[h][:, c0:c0 + cw],
                                  fts_d[h * 128:(h + 1) * 128, c0:c0 + cw])
            nc.sync.dma_start(pf[:, 8:520], pack_f_d[:, 8:520])
            nc.scalar.dma_start(pb[:, 512:1024], pack_b_d[:, 512:1024])
            nc.gpsimd.dma_start(pb[:, 1024:1664], pack_b_d[:, 1024:1664])
            nc.gpsimd.dma_start(pf[:, 520:1288], pack_f_d[:, 520:1288])
            W_ihT_sb = {}
            for h in range(2):
                t = cp.tile([128, 768], f32r, tag=f"wih{h}")
                nc.gpsimd.dma_start(t[:], W_ihT_d[h * 128:(h + 1) * 128, :])
                W_ihT_sb[h] = t
            # pack slices (views)
            W_rT_sb = {(r, h): pb[:, (r * 2 + h) * 256:(r * 2 + h + 1) * 256]
                       for r in range(NUM_REL) for h in range(2)}
            W1T_sb = {h: pb[:, 1024 + h * 128:1024 + (h + 1) * 128]
                      for h in range(2)}
            W2T_sb = pb[:, 1280:1408]
            W3T_sb = pb[:, 1408:1664]
            b_r_col_sb = {(r, h): pf[:, r * 2 + h:r * 2 + h + 1]
                          for r in range(NUM_REL) for h in range(2)}
            b1_col_sb = pf[:, 4:5]
            b2_col_sb = pf[:, 5:6]
            b3_col_sb = {h: pf[:, 6 + h:7 + h] for h in range(2)}
            b_r_rep_sb = {r: pf[:, 8 + r * 256:8 + (r + 1) * 256]
                          for r in range(NUM_REL)}
            b_g_rep_sb = pf[:, 520:1288]

            ident = cp.tile([128, 128], f32, tag="ident")
            make_identity(nc, ident[:])

            # ---- H table (full, gathered) and local shard staging ----
            hbuf = hp_pool.tile([128, NKT, D], bf16, tag="hbuf")
            hstage = hp_pool.tile([128, KT_PC, D], bf16, tag="hstage")

            # ---- phase A: local H shard (this core's src slice) ----
            # Own PSUM pool (closed before the agg accumulators open) so the
            # z1/hp pipelines can double-buffer across chunks.
            with tc.tile_pool(name="psA", bufs=1, space="PSUM") as psA:
                for r in range(NUM_REL):
                    for (c0, cw) in A_CHUNKS:
                        z1s = {}
                        for do_h in range(2):
                            z1p = psA.tile([128, 512], f32, tag="z1",
                                           space="PSUM", bufs=3)
                            for di_h in range(2):
                                nc.tensor.matmul(
                                    z1p[:, 0:cw],
                                    lhsT=W_rT_sb[r, di_h][
                                        :, do_h * 128:(do_h + 1) * 128],
                                    rhs=ft[di_h][:, c0:c0 + cw],
                                    start=(di_h == 0), stop=(di_h == 1))
                            z = wp.tile([128, 512], bf16, tag=f"z1s{do_h}")
                            nc.scalar.activation(z[:, 0:cw], z1p[:, 0:cw],
                                                 Relu,
                                                 bias=b_r_col_sb[r, do_h],
                                                 scale=1.0)
                            z1s[do_h] = z
                        for c4 in range(cw // 128):
                            lkt = r * KT_G + (c0 // 128) + c4
                            hp = psA.tile([128, D], f32, tag="hp",
                                          space="PSUM", bufs=4)
                            sl = slice(c4 * 128, (c4 + 1) * 128)
                            nc.tensor.matmul(hp[:], lhsT=z1s[0][:, sl],
                                             rhs=W_rT_sb[r, 0][:],
                                             start=True, stop=False)
                            nc.tensor.matmul(hp[:], lhsT=z1s[1][:, sl],
                                             rhs=W_rT_sb[r, 1][:],
                                             start=False, stop=True)
                            nc.vector.tensor_add(hp[:], hp[:],
                                                 b_r_rep_sb[r])
                            nc.scalar.activation(hstage[:, lkt, :], hp[:],
                                                 Relu, bias=0.0, scale=1.0)
                    # shard -> DRAM -> AllGather (Shared out) -> hbuf
                    g = r
                    nc.sync.dma_start(
                        shard_d[g][:, :, :],
                        hstage[:, g * KT_G:(g + 1) * KT_G, :])
                    nc.gpsimd.collective_compute(
                        "AllGather",
                        mybir.AluOpType.bypass,
                        replica_groups=[list(range(NCORES))],
                        ins=[shard_d[g][:, :, :].opt()],
                        outs=[gath_d[g][:, :, :, :].opt()],
                    )
                    for c in range(NCORES):
                        eng = [nc.sync, nc.scalar, nc.gpsimd][c % 3]
                        base = g * (NCORES * KT_G) + c * KT_G
                        eng.dma_start(
                            hbuf[:, base:base + KT_G, :],
                            gath_d[g][c, :, :, :])

            # agg psum accumulators
            with tc.tile_pool(name="psAgg", bufs=1, space="PSUM") as psAgg:
                aggp = {}
                for h in range(2):
                    for ci, (c0, cw) in enumerate(CHUNKS):
                        aggp[h, ci] = psAgg.tile([128, cw], f32,
                                                 tag=f"agg{h}{ci}",
                                                 space="PSUM",
                                                 name=f"agg{h}{ci}")

                # ---- agg: aggT[h] += H_k[:, h]^T @ C_k over all k ----
                for j in range(NKT // GK):
                    eng = nc.sync if j % 2 == 0 else nc.scalar
                    ct = wp.tile([128, GK, NDST], fp8, tag="ct", bufs=5)
                    eng.dma_start(ct[:], C_d[j, :, :, :])
                    for i in range(GK):
                        k = j * GK + i
                        for h in range(2):
                            for ci, (c0, cw) in enumerate(CHUNKS):
                                nc.tensor.matmul(
                                    aggp[h, ci][:],
                                    lhsT=hbuf[:, k,
                                              h * 128:(h + 1) * 128],
                                    rhs=ct[:, i, c0:c0 + cw],
                                    start=(k == 0), stop=(k == NKT - 1))

                # aggT in SBUF: [feat-half 128][NDST_PAD], f32 (used as f32r);
                # cols 1250:1280 stay uninitialized and feed only the unused
                # tail columns of the last phase-C block (never DMA'd out).
                aggT_sb = {}
                for h in range(2):
                    aggT_sb[h] = ap_pool.tile([128, NDST_PAD], f32r,
                                              tag=f"aggT{h}", name=f"aggT{h}")
                    for ci, (c0, cw) in enumerate(CHUNKS):
                        nc.vector.tensor_copy(aggT_sb[h][:, c0:c0 + cw],
                                              aggp[h, ci][:])

            # ---- phase C: LSTM (single step from zero state) + MLP ----
            with tc.tile_pool(name="psC", bufs=1, space="PSUM") as psC:
                for b in range(NBLK):
                    nn = min(128, NPC - b * 128)
                    bsl = slice(b * 128, (b + 1) * 128)
                    cbG = psC.tile([128, 512], f32, tag="cbG", space="PSUM",
                                   bufs=2)
                    cbT = psC.tile([128, 512], f32, tag="cbT", space="PSUM",
                                   bufs=2)
                    cbM = psC.tile([128, 512], f32, tag="cbM", space="PSUM",
                                   bufs=2)
                    # i gates in cbG[0:256]
                    for h in range(2):
                        nc.tensor.matmul(
                            cbG[:, 0:256],
                            lhsT=aggT_sb[h][:, bsl],
                            rhs=W_ihT_sb[h][:, 0:256],
                            start=(h == 0), stop=(h == 1))
                    nc.vector.tensor_add(cbG[:, 0:256], cbG[:, 0:256],
                                         b_g_rep_sb[:, 0:256])
                    si = wp.tile([128, 256], f32, tag="si")
                    nc.scalar.activation(si[:], cbG[:, 0:256], Sig,
                                         bias=0.0, scale=1.0)
                    # g gates in cbG[256:512]
                    for h in range(2):
                        nc.tensor.matmul(
                            cbG[:, 256:512],
                            lhsT=aggT_sb[h][:, bsl],
                            rhs=W_ihT_sb[h][:, 256:512],
                            start=(h == 0), stop=(h == 1))
                    nc.vector.tensor_add(cbG[:, 256:512],
                                         cbG[:, 256:512],
                                         b_g_rep_sb[:, 256:512])
                    tg = wp.tile([128, 256], f32, tag="tg")
                    nc.scalar.activation(tg[:], cbG[:, 256:512], Tanh,
                                         bias=0.0, scale=1.0)
                    # o gates reuse cbG[256:512] (tile-granular ordering
                    # serializes the reuse after tg's read)
                    for h in range(2):
                        nc.tensor.matmul(
                            cbG[:, 256:512],
                            lhsT=aggT_sb[h][:, bsl],
                            rhs=W_ihT_sb[h][:, 512:768],
                            start=(h == 0), stop=(h == 1))
                    nc.vector.tensor_add(cbG[:, 256:512],
                                         cbG[:, 256:512],
                                         b_g_rep_sb[:, 512:768])
                    so = wp.tile([128, 256], f32, tag="so")
                    nc.scalar.activation(so[:], cbG[:, 256:512], Sig,
                                         bias=0.0, scale=1.0)
                    cc = wp.tile([128, 256], f32, tag="cc")
                    nc.vector.tensor_mul(cc[:], si[:], tg[:])
                    tcc = wp.tile([128, 256], f32, tag="tcc")
                    nc.scalar.activation(tcc[:], cc[:], Tanh,
                                         bias=0.0, scale=1.0)
                    hn = wp.tile([128, 256], f32, tag="hn")
                    nc.vector.tensor_mul(hn[:], so[:], tcc[:])
                    hnT = {}
                    for h in range(2):
                        dst_sl = slice(h * 128, (h + 1) * 128)
                        nc.tensor.transpose(cbT[:, dst_sl],
                                            hn[:, h * 128:(h + 1) * 128],
                                            ident[:])
                        ht = wp.tile([128, 128], bf16, tag=f"hnT{h}")
                        nc.vector.tensor_copy(ht[:], cbT[:, dst_sl])
                        hnT[h] = ht
                    # MLP (transposed activation layout: [feature, node])
                    for h in range(2):
                        nc.tensor.matmul(cbM[:, 0:128], lhsT=W1T_sb[h],
                                         rhs=hnT[h][:],
                                         start=(h == 0), stop=(h == 1))
                    x1s = wp.tile([128, 128], bf16, tag="x1s")
                    nc.scalar.activation(x1s[:], cbM[:, 0:128], Relu,
                                         bias=b1_col_sb, scale=1.0)
                    nc.tensor.matmul(cbM[:, 128:256], lhsT=W2T_sb,
                                     rhs=x1s[:], start=True, stop=True)
                    x2s = wp.tile([128, 128], bf16, tag="x2s")
                    nc.scalar.activation(x2s[:], cbM[:, 128:256], Relu,
                                         bias=b2_col_sb, scale=1.0)
                    for oh in range(2):
                        nc.tensor.matmul(cbM[:, 256 + oh * 128:384 + oh * 128],
                                         lhsT=W3T_sb[:, oh * 128:(oh + 1) * 128],
                                         rhs=x2s[:], start=True, stop=True)
                        osb = wp.tile([128, 128], f32, tag=f"osb{oh}")
                        nc.vector.tensor_scalar_add(
                            osb[:], cbM[:, 256 + oh * 128:384 + oh * 128],
                            b3_col_sb[oh])
                        nc.gpsimd.dma_start(
                            outT_d[oh * 128:(oh + 1) * 128,
                                   b * 128:b * 128 + nn],
                            osb[:, 0:nn])

    nc.compile()
    return nc


_CACHE = {}


def _get_nc():
    if "nc" not in _CACHE:
        _CACHE["nc"] = _build()
    return _CACHE["nc"]


def prepare(inputs):
    """Build (nc, in_maps) for the SPMD run."""
    src = np.asarray(inputs["src"], dtype=np.int32)
    dst = np.asarray(inputs["dst"], dtype=np.int32)
    rel = np.asarray(inputs["rel"], dtype=np.int32)
    com, ft_slices = _prep_weights(inputs)
    Cs = _prep_counts(src, dst, rel)
    nc = _get_nc()
    in_maps = []
    for c in range(NCORES):
        m = dict(com)
        m["C"] = Cs[c]
        m["fts"] = ft_slices[c]
        in_maps.append(m)
    return nc, in_maps


# ----------------------------------------------------------------------------
# public entry
# ----------------------------------------------------------------------------

def kernel(**inputs) -> np.ndarray:
    nc, in_maps = prepare(inputs)
    res = bass_utils.run_bass_kernel_spmd(nc, in_maps,
                                          core_ids=list(range(NCORES)))
    out = np.empty((N_NODES, D_OUT), dtype=np.float32)
    for c in range(NCORES):
        out[c * NPC:(c + 1) * NPC, :] = res.results[c]["outT"].T
    return out


# revision 29
# speedup vs baseline: 1.0547x; 1.0547x over previous
"""Trainium2 Bass kernel for a relational GCN layer — src-sharded H + AllGather.

Math (reference):
  S = feat[src]; msgs[e] = edge_nn(S[e], W_rel[rel[e]]) (tied 2-layer relu MLP)
  agg = segment_sum(msgs, dst, N); hn = LSTM-step(agg); out = MLP(hn)

Messages depend only on (rel, src): H[r, s] = edge_nn(feat[s], W_rel[r]) has
NUM_REL*N = 20k rows << E = 320k.  agg[n] = sum_{r,s} C[n, (r,s)] * H[(r,s)]
where C is the per-node edge-count matrix (dense fp8, exact small ints).

v2 layout: phase A (the H table) is sharded by SOURCE range across the 8
cores (each core computes 2*1280 = 2560 rows = 8.6us of PE instead of the
8x-duplicated 69us), then the shards are exchanged with two AllGather
collectives (one per relation) through DRAM bounce buffers and re-loaded
into SBUF.  The aggregation itself stays dst-sharded exactly as before:
core c owns dst in [1250c, 1250(c+1)) and streams its dense count matrix
C [20480, 1250] from HBM while accumulating aggT = H^T @ C on the PE.
"""

import numpy as np
import ml_dtypes

import concourse.bacc as bacc
import concourse.bass as bass
import concourse.mybir as mybir
import concourse.tile as tile
from concourse import bass_utils
from concourse.masks import make_identity

# ---- problem constants (hardcoded per spec) ----
N_NODES = 10000
N_EDGES = 320000
D = 256
D_OUT = 256
NUM_REL = 2
NCORES = 8
NPC = N_NODES // NCORES          # 1250 dst nodes per core
NBLK = 10                        # dst blocks per core (last has 98 rows)
NDST = NPC                       # exact dst cols per core (no padding)
NPAD = 10240                     # src nodes padded to 8*1280
SRC_PC = NPAD // NCORES          # 1280 src rows per core
KT_PC = NUM_REL * SRC_PC // 128  # 20 local H k-tiles per core
KT_G = KT_PC // NUM_REL          # 10 k-tiles per relation group
NROWS = NUM_REL * NPAD           # 20480 H-table rows
NKT = NROWS // 128               # 160 k-tiles
GK = 4                           # k-tiles per C DMA tile (5000B lines)
CHUNKS = [(0, 512), (512, 512), (1024, 226)]  # dst col chunks (sum 1250)
A_CHUNKS = [(0, 512), (512, 512), (1024, 256)]  # src col chunks (sum 1280)
NDST_PAD = 1280                  # aggT sbuf width (phase C uses 128-blocks)

f32 = mybir.dt.float32
f32r = mybir.dt.float32r
bf16 = mybir.dt.bfloat16
fp8 = mybir.dt.float8e4

_np_bf16 = ml_dtypes.bfloat16
_np_fp8 = ml_dtypes.float8_e4m3


def _slot(r, s):
    """SBUF hbuf slot index for H row (r, s) after the gather."""
    core = s // SRC_PC
    lkt = (s % SRC_PC) // 128
    return r * (NCORES * KT_G) + core * KT_G + lkt


# ----------------------------------------------------------------------------
# host-side preprocessing
# ----------------------------------------------------------------------------

def _prep_counts(src, dst, rel):
    """Per-core dense count matrix in fp8, grouped GK k-tiles per DMA line.

    Returns per-core arrays of shape [NKT//GK, 128, GK, NDST].
    """
    r = rel.astype(np.int64)
    s = src.astype(np.int64)
    slot = r * (NCORES * KT_G) + (s // SRC_PC) * KT_G + (s % SRC_PC) // 128
    row = slot * 128 + (s % 128)
    core = dst // NPC
    col = (dst % NPC).astype(np.int64)
    counts = np.zeros((NCORES, NROWS, NDST), dtype=np.uint8)
    np.add.at(counts, (core, row, col), 1)
    out = []
    for c in range(NCORES):
        cc = counts[c].reshape(NKT // GK, GK, 128, NDST).transpose(0, 2, 1, 3)
        out.append(np.ascontiguousarray(cc).astype(_np_fp8))
    return out


def _prep_weights(inputs):
    feat = np.asarray(inputs["feat"], dtype=np.float32)
    W_rel = np.asarray(inputs["W_rel"], dtype=np.float32)
    b_rel = np.asarray(inputs["b_rel"], dtype=np.float32)
    W_ih = np.asarray(inputs["W_ih"], dtype=np.float32)
    b_ih = np.asarray(inputs["b_ih"], dtype=np.float32)
    b_hh = np.asarray(inputs["b_hh"], dtype=np.float32)
    W1 = np.asarray(inputs["W1"], dtype=np.float32)
    W2 = np.asarray(inputs["W2"], dtype=np.float32)
    W3 = np.asarray(inputs["W3"], dtype=np.float32)

    featT = np.zeros((D, NPAD), dtype=np.float32)
    featT[:, :N_NODES] = feat.T
    keep = np.r_[0:256, 512:1024]  # i, g, o gate columns (f unused: c0 = 0)

    # bf16 pack: W_rT (4x[128,256]) | W1T (2x[128,128]) | W2T | W3T [128,256]
    W_rT = np.transpose(W_rel, (0, 2, 1))  # [rel, in, out]
    pack_b = np.concatenate(
        [W_rT[0, 0:128], W_rT[0, 128:256], W_rT[1, 0:128], W_rT[1, 128:256],
         W1.T[0:128], W1.T[128:256], W2.T, W3.T], axis=1)  # [128, 1664]
    # f32 pack: b_r_col 4x[128,1] | b1|b2|b3a|b3b 4x[128,1] |
    #           b_r_rep 2x[128,256] | b_g_rep [128,768]
    b_r_col = b_rel.reshape(NUM_REL * 2, 128, 1)
    b3 = np.asarray(inputs["b3"]).reshape(2, 128, 1)
    pack_f = np.concatenate(
        [b_r_col[0], b_r_col[1], b_r_col[2], b_r_col[3],
         np.asarray(inputs["b1"]).reshape(128, 1),
         np.asarray(inputs["b2"]).reshape(128, 1), b3[0], b3[1],
         np.broadcast_to(b_rel[0][None, :], (128, D)),
         np.broadcast_to(b_rel[1][None, :], (128, D)),
         np.broadcast_to((b_ih + b_hh)[keep][None, :], (128, 768))],
        axis=1).astype(np.float32)  # [128, 1288]

    com = {
        "pack_b": np.ascontiguousarray(pack_b).astype(_np_bf16),
        "pack_f": np.ascontiguousarray(pack_f),
        "W_ihT": np.ascontiguousarray(W_ih.T[:, keep]).astype(np.float32),
    }
    featT_b = featT.astype(_np_bf16)
    slices = []
    for c in range(NCORES):
        slices.append(np.ascontiguousarray(
            featT_b[:, c * SRC_PC:(c + 1) * SRC_PC]))
    return com, slices


# ----------------------------------------------------------------------------
# kernel builder
# ----------------------------------------------------------------------------

def _build():
    Relu = mybir.ActivationFunctionType.Relu
    Sig = mybir.ActivationFunctionType.Sigmoid
    Tanh = mybir.ActivationFunctionType.Tanh

    nc = bacc.Bacc("TRN2", target_bir_lowering=False, debug=False)

    fts_d = nc.dram_tensor("fts", [D, SRC_PC], bf16, kind="ExternalInput")
    pack_b_d = nc.dram_tensor("pack_b", [128, 1664], bf16, kind="ExternalInput")
    pack_f_d = nc.dram_tensor("pack_f", [128, 1288], f32, kind="ExternalInput")
    W_ihT_d = nc.dram_tensor("W_ihT", [D, 768], f32r, kind="ExternalInput")
    C_d = nc.dram_tensor("C", [NKT // GK, 128, GK, NDST], fp8,
                         kind="ExternalInput")
    outT_d = nc.dram_tensor("outT", [D_OUT, NPC], f32, kind="ExternalOutput")

    with tile.TileContext(nc) as tc:
        with (
            tc.tile_pool(name="const", bufs=1) as cp,
            tc.tile_pool(name="work", bufs=3) as wp,
            tc.tile_pool(name="hbig", bufs=1) as hp_pool,
            tc.tile_pool(name="aggpool", bufs=1) as ap_pool,
            tc.tile_pool(name="dram", bufs=1, space="DRAM") as dram,
        ):
            # DRAM bounce buffers for the H-shard exchange; the gather
            # output is Shared so the AllGather takes the fast direct-write
            # path instead of the slow mesh algorithm.
            shard_d = [dram.tile([128, KT_G, D], bf16, tag=f"shard{g}",
                                 name=f"shard{g}")
                       for g in range(NUM_REL)]
            gath_d = [dram.tile([NCORES, 128, KT_G, D], bf16, tag=f"gath{g}",
                                name=f"gath{g}", addr_space="Shared")
                      for g in range(NUM_REL)]
            # ---- constants, ordered so phase A's first chunk starts ASAP:
            # W_r0 weights + first feat chunk on the two HW queues first.
            pb = cp.tile([128, 1664], bf16, tag="pb")
            nc.sync.dma_start(pb[:, 0:512], pack_b_d[:, 0:512])
            pf = cp.tile([128, 1288], f32, tag="pf")
            nc.scalar.dma_start(pf[:, 0:8], pack_f_d[:, 0:8])
            ft = {}
            for h in range(2):
                t = cp.tile([128, SRC_PC], bf16, tag=f"ft{h}")
                ft[h] = t
            for ci, (c0, cw) in enumerate(A_CHUNKS):
                for h in range(2):
                    eng = nc.sync if (ci * 2 + h) % 2 == 0 else nc.scalar
                    eng.dma_start(ft[h][:, c0:c0 + cw],
                                  fts_d[h * 128:(h + 1) * 128, c0:c0 + cw])
            nc.sync.dma_start(pf[:, 8:520], pack_f_d[:, 8:520])
            nc.scalar.dma_start(pb[:, 512:1024], pack_b_d[:, 512:1024])
            nc.gpsimd.dma_start(pb[:, 1024:1664], pack_b_d[:, 1024:1664])
            nc.gpsimd.dma_start(pf[:, 520:1288], pack_f_d[:, 520:1288])
            W_ihT_sb = {}
            for h in range(2):
                t = cp.tile([128, 768], f32r, tag=f"wih{h}")
                nc.gpsimd.dma_start(t[:], W_ihT_d[h * 128:(h + 1) * 128, :])
                W_ihT_sb[h] = t
            # pack slices (views)
            W_rT_sb = {(r, h): pb[:, (r * 2 + h) * 256:(r * 2 + h + 1) * 256]
                       for r in range(NUM_REL) for h in range(2)}
            W1T_sb = {h: pb[:, 1024 + h * 128:1024 + (h + 1) * 128]
                      for h in range(2)}
            W2T_sb = pb[:, 1280:1408]
            W3T_sb = pb[:, 1408:1664]
            b_r_col_sb = {(r, h): pf[:, r * 2 + h:r * 2 + h + 1]
                          for r in range(NUM_REL) for h in range(2)}
            b1_col_sb = pf[:, 4:5]
            b2_col_sb = pf[:, 5:6]
            b3_col_sb = {h: pf[:, 6 + h:7 + h] for h in range(2)}
            b_r_rep_sb = {r: pf[:, 8 + r * 256:8 + (r + 1) * 256]
                          for r in range(NUM_REL)}
            b_g_rep_sb = pf[:, 520:1288]

            ident = cp.tile([128, 128], f32, tag="ident")
            make_identity(nc, ident[:])

            # ---- H table (full, gathered) and local shard staging ----
            hbuf = hp_pool.tile([128, NKT, D], bf16, tag="hbuf")
            hstage = hp_pool.tile([128, KT_PC, D], bf16, tag="hstage")

            # ---- phase A: local H shard (this core's src slice) ----
            # Own PSUM pool (closed before the agg accumulators open) so the
            # z1/hp pipelines can double-buffer across chunks.
            with tc.tile_pool(name="psA", bufs=1, space="PSUM") as psA:
                for r in range(NUM_REL):
                    for (c0, cw) in A_CHUNKS:
                        z1s = {}
                        for do_h in range(2):
                            z1p = psA.tile([128, 512], f32, tag="z1",
                                           space="PSUM", bufs=3)
                            for di_h in range(2):
                                nc.tensor.matmul(
                                    z1p[:, 0:cw],
                                    lhsT=W_rT_sb[r, di_h][
                                        :, do_h * 128:(do_h + 1) * 128],
                                    rhs=ft[di_h][:, c0:c0 + cw],
                                    start=(di_h == 0), stop=(di_h == 1))
                            z = wp.tile([128, 512], bf16, tag=f"z1s{do_h}")
                            nc.scalar.activation(z[:, 0:cw], z1p[:, 0:cw],
                                                 Relu,
                                                 bias=b_r_col_sb[r, do_h],
                                                 scale=1.0)
                            z1s[do_h] = z
                        for c4 in range(cw // 128):
                            lkt = r * KT_G + (c0 // 128) + c4
                            hp = psA.tile([128, D], f32, tag="hp",
                                          space="PSUM", bufs=4)
                            sl = slice(c4 * 128, (c4 + 1) * 128)
                            nc.tensor.matmul(hp[:], lhsT=z1s[0][:, sl],
                                             rhs=W_rT_sb[r, 0][:],
                                             start=True, stop=False)
                            nc.tensor.matmul(hp[:], lhsT=z1s[1][:, sl],
                                             rhs=W_rT_sb[r, 1][:],
                                             start=False, stop=True)
                            nc.vector.tensor_add(hp[:], hp[:],
                                                 b_r_rep_sb[r])
                            nc.scalar.activation(hstage[:, lkt, :], hp[:],
                                                 Relu, bias=0.0, scale=1.0)
                    # shard -> DRAM -> AllGather (Shared out) -> hbuf
                    g = r
                    nc.sync.dma_start(
                        shard_d[g][:, :, :],
                        hstage[:, g * KT_G:(g + 1) * KT_G, :])
                    nc.gpsimd.collective_compute(
                        "AllGather",
                        mybir.AluOpType.bypass,
                        replica_groups=[list(range(NCORES))],
                        ins=[shard_d[g][:, :, :].opt()],
                        outs=[gath_d[g][:, :, :, :].opt()],
                    )
                    for c in range(NCORES):
                        eng = [nc.sync, nc.scalar, nc.gpsimd][c % 3]
                        base = g * (NCORES * KT_G) + c * KT_G
                        eng.dma_start(
                            hbuf[:, base:base + KT_G, :],
                            gath_d[g][c, :, :, :])

            # agg psum accumulators
            with tc.tile_pool(name="psAgg", bufs=1, space="PSUM") as psAgg:
                aggp = {}
                for h in range(2):
                    for ci, (c0, cw) in enumerate(CHUNKS):
                        aggp[h, ci] = psAgg.tile([128, cw], f32,
                                                 tag=f"agg{h}{ci}",
                                                 space="PSUM",
                                                 name=f"agg{h}{ci}")

                # ---- agg: aggT[h] += H_k[:, h]^T @ C_k over all k ----
                for j in range(NKT // GK):
                    eng = nc.sync if j % 2 == 0 else nc.scalar
                    ct = wp.tile([128, GK, NDST], fp8, tag="ct", bufs=5)
                    eng.dma_start(ct[:], C_d[j, :, :, :])
                    for i in range(GK):
                        k = j * GK + i
                        for h in range(2):
                            for ci, (c0, cw) in enumerate(CHUNKS):
                                nc.tensor.matmul(
                                    aggp[h, ci][:],
                                    lhsT=hbuf[:, k,
                                              h * 128:(h + 1) * 128],
                                    rhs=ct[:, i, c0:c0 + cw],
                                    start=(k == 0), stop=(k == NKT - 1))

                # aggT in SBUF: [feat-half 128][NDST_PAD], f32 (used as f32r);
                # cols 1250:1280 stay uninitialized and feed only the unused
                # tail columns of the last phase-C block (never DMA'd out).
                aggT_sb = {}
                for h in range(2):
                    aggT_sb[h] = ap_pool.tile([128, NDST_PAD], f32r,
                                              tag=f"aggT{h}", name=f"aggT{h}")
                    for ci, (c0, cw) in enumerate(CHUNKS):
                        nc.vector.tensor_copy(aggT_sb[h][:, c0:c0 + cw],
                                              aggp[h, ci][:])

            # ---- phase C: LSTM (single step from zero state) + MLP ----
            with tc.tile_pool(name="psC", bufs=1, space="PSUM") as psC:
                for b in range(NBLK):
                    nn = min(128, NPC - b * 128)
                    bsl = slice(b * 128, (b + 1) * 128)
                    cbG = psC.tile([128, 512], f32, tag="cbG", space="PSUM",
                                   bufs=2)
                    cbT = psC.tile([128, 512], f32, tag="cbT", space="PSUM",
                                   bufs=2)
                    cbM = psC.tile([128, 512], f32, tag="cbM", space="PSUM",
                                   bufs=2)
                    # i gates in cbG[0:256]
                    for h in range(2):
                        nc.tensor.matmul(
                            cbG[:, 0:256],
                            lhsT=aggT_sb[h][:, bsl],
                            rhs=W_ihT_sb[h][:, 0:256],
                            start=(h == 0), stop=(h == 1))
                    nc.vector.tensor_add(cbG[:, 0:256], cbG[:, 0:256],
                                         b_g_rep_sb[:, 0:256])
                    si = wp.tile([128, 256], f32, tag="si")
                    nc.scalar.activation(si[:], cbG[:, 0:256], Sig,
                                         bias=0.0, scale=1.0)
                    # g gates in cbG[256:512]
                    for h in range(2):
                        nc.tensor.matmul(
                            cbG[:, 256:512],
                            lhsT=aggT_sb[h][:, bsl],
                            rhs=W_ihT_sb[h][:, 256:512],
                            start=(h == 0), stop=(h == 1))
                    nc.vector.tensor_add(cbG[:, 256:512],
                                         cbG[:, 256:512],
                                         b_g_rep_sb[:, 256:512])
                    tg = wp.tile([128, 256], f32, tag="tg")
                    nc.scalar.activation(tg[:], cbG[:, 256:512], Tanh,
                                         bias=0.0, scale=1.0)
                    # o gates reuse cbG[256:512] (tile-granular ordering
                    # serializes the reuse after tg's read)
                    for h in range(2):
                        nc.tensor.matmul(
                            cbG[:, 256:512],
                            lhsT=aggT_sb[h][:, bsl],
                            rhs=W_ihT_sb[h][:, 512:768],
                            start=(h == 0), stop=(h == 1))
                    nc.vector.tensor_add(cbG[:, 256:512],
                                         cbG[:, 256:512],
                                         b_g_rep_sb[:, 512:768])
                    so = wp.tile([128, 256], f32, tag="so")
                    nc.scalar.activation(so[:], cbG[:, 256:512], Sig,
                                         bias=0.0, scale=1.0)
                    cc = wp.tile([128, 256], f32, tag="cc")
                    nc.vector.tensor_mul(cc[:], si[:], tg[:])
                    tcc = wp.tile([128, 256], f32, tag="tcc")
                    nc.scalar.activation(tcc[:], cc[:], Tanh,
                                         bias=0.0, scale=1.0)
                    hn = wp.tile([128, 256], f32, tag="hn")
                    nc.vector.tensor_mul(hn[:], so[:], tcc[:])
                    hnT = {}
                    for h in range(2):
                        dst_sl = slice(h * 128, (h + 1) * 128)
                        nc.tensor.transpose(cbT[:, dst_sl],
                                            hn[:, h * 128:(h + 1) * 128],
                                            ident[:])
                        ht = wp.tile([128, 128], bf16, tag=f"hnT{h}")
                        nc.vector.tensor_copy(ht[:], cbT[:, dst_sl])
                        hnT[h] = ht
                    # MLP (transposed activation layout: [feature, node])
                    for h in range(2):
                        nc.tensor.matmul(cbM[:, 0:128], lhsT=W1T_sb[h],
                                         rhs=hnT[h][:],
                                         start=(h == 0), stop=(h == 1))
                    x1s = wp.tile([128, 128], bf16, tag="x1s")
                    nc.scalar.activation(x1s[:], cbM[:, 0:128], Relu,
                                         bias=b1_col_sb, scale=1.0)
                    nc.tensor.matmul(cbM[:, 128:256], lhsT=W2T_sb,
                                     rhs=x1s[:], start=True, stop=True)
                    x2s = wp.tile([128, 128], bf16, tag="x2s")
                    nc.scalar.activation(x2s[:], cbM[:, 128:256], Relu,
                                         bias=b2_col_sb, scale=1.0)
                    for oh in range(2):
                        nc.tensor.matmul(cbM[:, 256 + oh * 128:384 + oh * 128],
                                         lhsT=W3T_sb[:, oh * 128:(oh + 1) * 128],
                                         rhs=x2s[:], start=True, stop=True)
                        osb = wp.tile([128, 128], f32, tag=f"osb{oh}")
                        nc.vector.tensor_scalar_add(
                            osb[:], cbM[:, 256 + oh * 128:384 + oh * 128],
                            b3_col_sb[oh])
                        nc.gpsimd.dma_start(
                            outT_d[oh * 128:(oh + 1) * 128,
                                   b * 128:b * 128 + nn],
                            osb[:, 0:nn])

    nc.compile()
    return nc


_CACHE = {}


def _get_nc():
    if "nc" not in _CACHE:
        _CACHE["nc"] = _build()
    return _CACHE["nc"]


def prepare(inputs):
    """Build (nc, in_maps) for the SPMD run."""
    src = np.asarray(inputs["src"], dtype=np.int32)
    dst = np.asarray(inputs["dst"], dtype=np.int32)
    rel = np.asarray(inputs["rel"], dtype=np.int32)
    com, ft_slices = _prep_weights(inputs)
    Cs = _prep_counts(src, dst, rel)
    nc = _get_nc()
    in_maps = []
    for c in range(NCORES):
        m = dict(com)
        m["C"] = Cs[c]
        m["fts"] = ft_slices[c]
        in_maps.append(m)
    return nc, in_maps


# ----------------------------------------------------------------------------
# public entry
# ----------------------------------------------------------------------------

def kernel(**inputs) -> np.ndarray:
    nc, in_maps = prepare(inputs)
    res = bass_utils.run_bass_kernel_spmd(nc, in_maps,
                                          core_ids=list(range(NCORES)))
    out = np.empty((N_NODES, D_OUT), dtype=np.float32)
    for c in range(NCORES):
        out[c * NPC:(c + 1) * NPC, :] = res.results[c]["outT"].T
    return out
